# revision 3
# baseline (speedup 1.0000x reference)
"""Causal single-head attention on 8 TRN2 NeuronCores.

Problem: x [4, 4096, 1024] f32, Wq/Wk/Wv [1024, 128] f32 ->
out [4, 4096, 128] f32 (causal softmax(QK^T/sqrt(128)) V).

Sharding: 2 cores per batch element; within a batch element the two
cores take interleaved query rows (even / odd). Each q tile of 128
interleaved rows spans 256 original rows, so tile j needs the kv prefix
0..256(j+1) — identical structure on every core (SPMD), causal waste
only ~6%.

Per-core kernel (all matmuls bf16, fp32 PSUM accumulation):
  K.T [h, seq]  = sum_c Wk_c.T @ xT_c            (c = 8 d_model chunks)
  V   [seq, h+1]= sum_c xT_c,t.T @ Wv_c, ones col appended
  Q.T [h, 2048] = sum_c Wq'_c.T @ xqT_c          (Wq' pre-scaled 1/sqrt(h))
  per q tile j: S.T chunks = (K.T chunk).T @ Q.T tile (PSUM),
  P.T = exp(S.T) (ScalarE, no max subtraction - scores are O(5)),
  0/1 mask multiply on the two diagonal chunks,
  out tile = (P.T chunk).T @ [V | 1] accumulated over chunks (PSUM);
  last column is the softmax denominator; DVE reciprocal + scale.
"""
import sys

if "/opt/trn_rl_repo" not in sys.path:
    sys.path.insert(0, "/opt/trn_rl_repo")

import numpy as np
import ml_dtypes

SEQ, D, H = 4096, 1024, 128
NQ = SEQ // 2        # q rows per core
NT = NQ // 128       # q tiles per core (16)
DC = D // 128        # d_model chunks (8)
SB = 512             # seq block for K/Q projections
N_CORES = 8

_BF16 = ml_dtypes.bfloat16
_CACHED_NC = None


def _build_nc():
    import concourse.bass as bass
    import concourse.bacc as bacc
    import concourse.mybir as mybir
    import concourse.tile as tile

    bf = mybir.dt.bfloat16
    f32 = mybir.dt.float32
    Exp = mybir.ActivationFunctionType.Exp

    nc = bacc.Bacc("TRN2", target_bir_lowering=False, debug=False,
                   num_devices=N_CORES)

    xT_d = nc.dram_tensor("xT", [D, SEQ], bf, kind="ExternalInput")
    xqT_d = nc.dram_tensor("xqT", [D, NQ], bf, kind="ExternalInput")
    wq_d = nc.dram_tensor("wq", [D, H], bf, kind="ExternalInput")
    wk_d = nc.dram_tensor("wk", [D, H], bf, kind="ExternalInput")
    wv_d = nc.dram_tensor("wv", [D, H], bf, kind="ExternalInput")
    m0_d = nc.dram_tensor("mask0", [128, 128], bf, kind="ExternalInput")
    m1_d = nc.dram_tensor("mask1", [128, 128], bf, kind="ExternalInput")
    out_d = nc.dram_tensor("out", [NQ, H], f32, kind="ExternalOutput")

    with tile.TileContext(nc) as tc:
        with (
            tc.tile_pool(name="const", bufs=1) as constp,
            tc.tile_pool(name="xbuf", bufs=1) as xbuf,
            tc.tile_pool(name="acts", bufs=1) as acts,
            tc.tile_pool(name="pt", bufs=3) as ptp,
            tc.tile_pool(name="ob", bufs=2) as obp,
            tc.tile_pool(name="rc", bufs=2) as rcp,
        ):
            # ---- constants ----
            wq_sb = constp.tile([128, DC, H], bf)
            wk_sb = constp.tile([128, DC, H], bf)
            wv_sb = constp.tile([128, DC, H], bf)
            for c in range(DC):
                nc.sync.dma_start(wq_sb[:, c, :], wq_d[128 * c:128 * (c + 1), :])
                nc.sync.dma_start(wk_sb[:, c, :], wk_d[128 * c:128 * (c + 1), :])
                nc.sync.dma_start(wv_sb[:, c, :], wv_d[128 * c:128 * (c + 1), :])
            m0_sb = constp.tile([128, 128], bf)
            m1_sb = constp.tile([128, 128], bf)
            nc.sync.dma_start(m0_sb[:], m0_d[:, :])
            nc.sync.dma_start(m1_sb[:], m1_d[:, :])

            # ---- x in SBUF ----
            xT = xbuf.tile([128, DC, SEQ], bf)
            xqT = xbuf.tile([128, DC, NQ], bf)

            # ---- projection outputs ----
            KT = acts.tile([128, SEQ], bf)
            QT = acts.tile([128, NQ], bf)
            V = acts.tile([128, SEQ // 128, H + 1], bf)
            nc.vector.memset(V[:, :, H:H + 1], 1.0)

            # ---- phases 1-3: stream x in seq blocks, project ----
            with (
                tc.tile_pool(name="psum_kq", bufs=3, space="PSUM") as psum_kq,
                tc.tile_pool(name="psum_v", bufs=3, space="PSUM") as psum_v,
            ):
                for s in range(SEQ // SB):
                    lo = SB * s
                    for c in range(DC):
                        nc.sync.dma_start(
                            xT[:, c, lo:lo + SB],
                            xT_d[128 * c:128 * (c + 1), lo:lo + SB])
                    # K.T block
                    pk = psum_kq.tile([128, SB], f32, tag="kq")
                    for c in range(DC):
                        nc.tensor.matmul(pk[:], wk_sb[:, c, :],
                                         xT[:, c, lo:lo + SB],
                                         start=(c == 0), stop=(c == DC - 1))
                    nc.vector.tensor_copy(KT[:, lo:lo + SB], pk[:])
                    # V blocks (4 seq tiles of 128 per block)
                    for t in range(4 * s, 4 * s + 4):
                        pv = psum_v.tile([128, H], f32, tag="v")
                        for c in range(DC):
                            nc.tensor.matmul(pv[:],
                                             xT[:, c, 128 * t:128 * (t + 1)],
                                             wv_sb[:, c, :],
                                             start=(c == 0), stop=(c == DC - 1))
                        nc.vector.tensor_copy(V[:, t, 0:H], pv[:])
                    # Q.T block (first 4 s-blocks cover NQ=2048)
                    if lo < NQ:
                        for c in range(DC):
                            nc.sync.dma_start(
                                xqT[:, c, lo:lo + SB],
                                xqT_d[128 * c:128 * (c + 1), lo:lo + SB])
                        pq = psum_kq.tile([128, SB], f32, tag="kq")
                        for c in range(DC):
                            nc.tensor.matmul(pq[:], wq_sb[:, c, :],
                                             xqT[:, c, lo:lo + SB],
                                             start=(c == 0), stop=(c == DC - 1))
                        nc.vector.tensor_copy(QT[:, lo:lo + SB], pq[:])

            # ---- phase 4: attention ----
            with (
                tc.tile_pool(name="psum_s", bufs=2, space="PSUM") as psum_s,
                tc.tile_pool(name="psum_o", bufs=2, space="PSUM") as psum_o,
            ):
              for j in range(NT):
                Cj = 2 * (j + 1)
                po = psum_o.tile([128, H + 1], f32)
                for g in range(0, Cj, 8):
                    gn = min(8, Cj - g)
                    ps = psum_s.tile([128, 1024], f32)
                    for k in range(gn):
                        c = g + k
                        nc.tensor.matmul(ps[:, 128 * k:128 * (k + 1)],
                                         KT[:, 128 * c:128 * (c + 1)],
                                         QT[:, 128 * j:128 * (j + 1)],
                                         start=True, stop=True)
                    pt = ptp.tile([128, 1024], bf)
                    nc.scalar.activation(pt[:, 0:128 * gn], ps[:, 0:128 * gn], Exp)
                    for k in range(gn):
                        c = g + k
                        if c == Cj - 2:
                            nc.vector.tensor_mul(pt[:, 128 * k:128 * (k + 1)],
                                                 pt[:, 128 * k:128 * (k + 1)],
                                                 m0_sb[:])
                        elif c == Cj - 1:
                            nc.vector.tensor_mul(pt[:, 128 * k:128 * (k + 1)],
                                                 pt[:, 128 * k:128 * (k + 1)],
                                                 m1_sb[:])
                    for k in range(gn):
                        c = g + k
                        nc.tensor.matmul(po[:], pt[:, 128 * k:128 * (k + 1)],
                                         V[:, c, :],
                                         start=(c == 0), stop=(c == Cj - 1),
                                         skip_group_check=True)
                rc = rcp.tile([128, 1], f32)
                nc.vector.reciprocal(rc[:], po[:, H:H + 1])
                ob = obp.tile([128, H], f32)
                nc.vector.tensor_scalar_mul(ob[:], po[:, 0:H], rc[:])
                nc.sync.dma_start(out_d[128 * j:128 * (j + 1), :], ob[:])

    nc.compile()
    return nc


def _get_nc():
    global _CACHED_NC
    if _CACHED_NC is None:
        _CACHED_NC = _build_nc()
    return _CACHED_NC


def _make_in_maps(x, Wq, Wk, Wv):
    scale = np.float32(1.0 / np.sqrt(H))
    wq = (np.asarray(Wq, np.float32) * scale).astype(_BF16)
    wk = np.asarray(Wk, np.float32).astype(_BF16)
    wv = np.asarray(Wv, np.float32).astype(_BF16)
    cc, ii = np.meshgrid(np.arange(128), np.arange(128), indexing="ij")
    masks = [
        ((cc <= 2 * ii + p).astype(np.float32).astype(_BF16),
         (cc + 128 <= 2 * ii + p).astype(np.float32).astype(_BF16))
        for p in (0, 1)
    ]
    xTs = [np.ascontiguousarray(np.asarray(x[b], np.float32).T).astype(_BF16)
           for b in range(4)]
    in_maps = []
    for core in range(N_CORES):
        b, p = core // 2, core % 2
        xqT = np.ascontiguousarray(np.asarray(x[b, p::2], np.float32).T).astype(_BF16)
        in_maps.append({
            "xT": xTs[b], "xqT": xqT,
            "wq": wq, "wk": wk, "wv": wv,
            "mask0": masks[p][0], "mask1": masks[p][1],
        })
    return in_maps


def _run(x, Wq, Wk, Wv, trace=False):
    from concourse import bass_utils
    nc = _get_nc()
    in_maps = _make_in_maps(x, Wq, Wk, Wv)
    res = bass_utils.run_bass_kernel_spmd(
        nc, in_maps, core_ids=list(range(N_CORES)), trace=trace)
    out = np.empty((4, SEQ, H), np.float32)
    for core in range(N_CORES):
        out[core // 2, core % 2::2] = res.results[core]["out"]
    return out, res


def kernel(x, Wq, Wk, Wv):
    out, _ = _run(x, Wq, Wk, Wv, trace=False)
    return out


# revision 4
# speedup vs baseline: 1.1347x; 1.1347x over previous
"""Causal single-head attention on 8 TRN2 NeuronCores.

Problem: x [4, 4096, 1024] f32, Wq/Wk/Wv [1024, 128] f32 ->
out [4, 4096, 128] f32 (causal softmax(QK^T/sqrt(128)) V).

Sharding: 2 cores per batch element; within a batch element the two
cores take interleaved query rows (even / odd). Each q tile of 128
interleaved rows spans 256 original rows, so tile j needs the kv prefix
0..256(j+1) — identical structure on every core (SPMD), causal waste
only ~6%.

Per-core kernel (all matmuls bf16, fp32 PSUM accumulation):
  K.T [h, seq]  = sum_c Wk_c.T @ xT_c   (chunk-major, tracks xT DMA)
  Q.T [h, 2048] = sum_c Wq'_c.T @ xqT_c (Wq' pre-scaled by 1/sqrt(h))
  V   [seq, h+1]= sum_c xT_c,t.T @ Wv_c, ones col; interleaved with attn
  per q tile j: S.T chunks = (K.T chunk).T @ Q.T tile (PSUM),
  P.T = exp(S.T) (ScalarE, no max subtraction - scores are O(5)),
  0/1 mask multiply on the two diagonal chunks,
  out tile = (P.T chunk).T @ [V | 1] accumulated over chunks (PSUM);
  last column is the softmax denominator; DVE reciprocal + scale.

DMA: weights/masks host-packed to match SBUF layout (single contiguous
transfers); xT split per d-chunk (1 MB each) alternating between the
two HWDGE rings (sync / scalar); xqT via SWDGE (gpsimd); outputs on the
scalar ring.
"""
import sys

if "/opt/trn_rl_repo" not in sys.path:
    sys.path.insert(0, "/opt/trn_rl_repo")

import numpy as np
import ml_dtypes

SEQ, D, H = 4096, 1024, 128
NQ = SEQ // 2        # q rows per core
NT = NQ // 128       # q tiles per core (16)
DC = D // 128        # d_model chunks (8)
SB = 512             # seq block for K/Q psum regions
N_CORES = 8

_BF16 = ml_dtypes.bfloat16
_CACHED_NC = None


def _build_nc():
    import concourse.bass as bass
    import concourse.bacc as bacc
    import concourse.mybir as mybir
    import concourse.tile as tile

    bf = mybir.dt.bfloat16
    f32 = mybir.dt.float32
    Exp = mybir.ActivationFunctionType.Exp

    nc = bacc.Bacc("TRN2", target_bir_lowering=False, debug=False,
                   num_devices=N_CORES)

    xT_d = nc.dram_tensor("xT", [D, SEQ], bf, kind="ExternalInput")
    xqT_d = nc.dram_tensor("xqT", [D, NQ], bf, kind="ExternalInput")
    # weights packed on host to [128, DC*H] (partition-major chunks)
    wq_d = nc.dram_tensor("wq", [128, DC * H], bf, kind="ExternalInput")
    wk_d = nc.dram_tensor("wk", [128, DC * H], bf, kind="ExternalInput")
    wv_d = nc.dram_tensor("wv", [128, DC * H], bf, kind="ExternalInput")
    m_d = nc.dram_tensor("masks", [128, 256], bf, kind="ExternalInput")
    out_d = nc.dram_tensor("out", [NQ, H], f32, kind="ExternalOutput")

    with tile.TileContext(nc) as tc:
        with (
            tc.tile_pool(name="const", bufs=1) as constp,
            tc.tile_pool(name="xbuf", bufs=1) as xbuf,
            tc.tile_pool(name="acts", bufs=1) as acts,
            tc.tile_pool(name="pt", bufs=3) as ptp,
            tc.tile_pool(name="ob", bufs=2) as obp,
            tc.tile_pool(name="rc", bufs=4) as rcp,
        ):
            # ---- constants (scalar ring; sync ring reserved for xT) ----
            m_sb = constp.tile([128, 256], bf)
            nc.scalar.dma_start(m_sb[:], m_d[:, :])
            wq_sb = constp.tile([128, DC, H], bf)
            wk_sb = constp.tile([128, DC, H], bf)
            wv_sb = constp.tile([128, DC, H], bf)
            nc.scalar.dma_start(wk_sb[:], wk_d[:, :])
            nc.scalar.dma_start(wq_sb[:], wq_d[:, :])
            nc.scalar.dma_start(wv_sb[:], wv_d[:, :])
            m0 = m_sb[:, 0:128]
            m1 = m_sb[:, 128:256]

            # preload the ACT exp table while DMAs stream
            warm = rcp.tile([128, 1], f32)
            nc.scalar.activation(warm[:], m_sb[:, 0:1], Exp)

            # ---- x in SBUF ----
            xT = xbuf.tile([128, DC, SEQ], bf)
            xqT = xbuf.tile([128, DC, NQ], bf)
            for c in range(DC):
                eng = nc.sync if c % 2 == 0 else nc.scalar
                eng.dma_start(xT[:, c, :], xT_d[128 * c:128 * (c + 1), :])
            for c in range(DC):
                nc.gpsimd.dma_start(xqT[:, c, :], xqT_d[128 * c:128 * (c + 1), :])

            # ---- projection outputs ----
            KT = acts.tile([128, SEQ], bf)
            QT = acts.tile([128, NQ], bf)
            V = acts.tile([128, SEQ // 128, H + 1], bf)
            nc.vector.memset(V[:, :, H:H + 1], 1.0)

            # ---- K phase: chunk-major over full seq (8 PSUM banks) ----
            with tc.tile_pool(name="psum_k", bufs=1, space="PSUM") as psum_k:
                pk = psum_k.tile([128, SEQ], f32)
                for c in range(DC):
                    for s in range(SEQ // SB):
                        nc.tensor.matmul(pk[:, SB * s:SB * (s + 1)],
                                         wk_sb[:, c, :],
                                         xT[:, c, SB * s:SB * (s + 1)],
                                         start=(c == 0), stop=(c == DC - 1),
                                         skip_group_check=True)
                nc.vector.tensor_copy(KT[:, 0:SEQ // 2], pk[:, 0:SEQ // 2])
                nc.vector.tensor_copy(KT[:, SEQ // 2:SEQ], pk[:, SEQ // 2:SEQ])

            # ---- Q phase: chunk-major (4 PSUM banks) ----
            with tc.tile_pool(name="psum_q", bufs=1, space="PSUM") as psum_q:
                pq = psum_q.tile([128, NQ], f32)
                for c in range(DC):
                    for s in range(NQ // SB):
                        nc.tensor.matmul(pq[:, SB * s:SB * (s + 1)],
                                         wq_sb[:, c, :],
                                         xqT[:, c, SB * s:SB * (s + 1)],
                                         start=(c == 0), stop=(c == DC - 1),
                                         skip_group_check=True)
                nc.vector.tensor_copy(QT[:], pq[:])

            # ---- V projection interleaved with attention ----
            with (
                tc.tile_pool(name="psum_v", bufs=2, space="PSUM") as psum_v,
                tc.tile_pool(name="psum_s", bufs=2, space="PSUM") as psum_s,
                tc.tile_pool(name="psum_o", bufs=2, space="PSUM") as psum_o,
            ):
                for j in range(NT):
                    # V tiles 2j, 2j+1 (kv rows 256j .. 256j+256)
                    for t in (2 * j, 2 * j + 1):
                        pv = psum_v.tile([128, H], f32)
                        for c in range(DC):
                            nc.tensor.matmul(pv[:],
                                             xT[:, c, 128 * t:128 * (t + 1)],
                                             wv_sb[:, c, :],
                                             start=(c == 0), stop=(c == DC - 1))
                        nc.vector.tensor_copy(V[:, t, 0:H], pv[:])
                    # attention for q tile j
                    Cj = 2 * (j + 1)
                    po = psum_o.tile([128, H + 1], f32)
                    for g in range(0, Cj, 8):
                        gn = min(8, Cj - g)
                        ps = psum_s.tile([128, 1024], f32)
                        for k in range(gn):
                            c = g + k
                            nc.tensor.matmul(ps[:, 128 * k:128 * (k + 1)],
                                             KT[:, 128 * c:128 * (c + 1)],
                                             QT[:, 128 * j:128 * (j + 1)],
                                             start=True, stop=True)
                        pt = ptp.tile([128, 1024], bf)
                        nc.scalar.activation(pt[:, 0:128 * gn], ps[:, 0:128 * gn], Exp)
                        for k in range(gn):
                            c = g + k
                            if c == Cj - 2:
                                nc.vector.tensor_mul(pt[:, 128 * k:128 * (k + 1)],
                                                     pt[:, 128 * k:128 * (k + 1)],
                                                     m0)
                            elif c == Cj - 1:
                                nc.vector.tensor_mul(pt[:, 128 * k:128 * (k + 1)],
                                                     pt[:, 128 * k:128 * (k + 1)],
                                                     m1)
                        for k in range(gn):
                            c = g + k
                            nc.tensor.matmul(po[:], pt[:, 128 * k:128 * (k + 1)],
                                             V[:, c, :],
                                             start=(c == 0), stop=(c == Cj - 1),
                                             skip_group_check=True)
                    rc = rcp.tile([128, 1], f32)
                    nc.vector.reciprocal(rc[:], po[:, H:H + 1])
                    ob = obp.tile([128, H], f32)
                    nc.vector.tensor_scalar_mul(ob[:], po[:, 0:H], rc[:])
                    nc.scalar.dma_start(out_d[128 * j:128 * (j + 1), :], ob[:])

    nc.compile()
    return nc


def _get_nc():
    global _CACHED_NC
    if _CACHED_NC is None:
        _CACHED_NC = _build_nc()
    return _CACHED_NC


def _pack_w(w):
    # [1024, 128] -> [128, 8*128] so SBUF tile [128, c, h] is one linear DMA
    return np.ascontiguousarray(
        w.reshape(DC, 128, H).transpose(1, 0, 2).reshape(128, DC * H))


def _make_in_maps(x, Wq, Wk, Wv):
    scale = np.float32(1.0 / np.sqrt(H))
    wq = _pack_w((np.asarray(Wq, np.float32) * scale).astype(_BF16))
    wk = _pack_w(np.asarray(Wk, np.float32).astype(_BF16))
    wv = _pack_w(np.asarray(Wv, np.float32).astype(_BF16))
    cc, ii = np.meshgrid(np.arange(128), np.arange(128), indexing="ij")
    masks = []
    for p in (0, 1):
        m0 = (cc <= 2 * ii + p).astype(np.float32)
        m1 = (cc + 128 <= 2 * ii + p).astype(np.float32)
        masks.append(np.concatenate([m0, m1], axis=1).astype(_BF16))
    xTs = [np.ascontiguousarray(np.asarray(x[b], np.float32).T).astype(_BF16)
           for b in range(4)]
    in_maps = []
    for core in range(N_CORES):
        b, p = core // 2, core % 2
        xqT = np.ascontiguousarray(np.asarray(x[b, p::2], np.float32).T).astype(_BF16)
        in_maps.append({
            "xT": xTs[b], "xqT": xqT,
            "wq": wq, "wk": wk, "wv": wv,
            "masks": masks[p],
        })
    return in_maps


def _run(x, Wq, Wk, Wv, trace=False):
    from concourse import bass_utils
    nc = _get_nc()
    in_maps = _make_in_maps(x, Wq, Wk, Wv)
    res = bass_utils.run_bass_kernel_spmd(
        nc, in_maps, core_ids=list(range(N_CORES)), trace=trace)
    out = np.empty((4, SEQ, H), np.float32)
    for core in range(N_CORES):
        out[core // 2, core % 2::2] = res.results[core]["out"]
    return out, res


def kernel(x, Wq, Wk, Wv):
    out, _ = _run(x, Wq, Wk, Wv, trace=False)
    return out


# revision 6
# speedup vs baseline: 1.2639x; 1.1138x over previous
"""Causal single-head attention on 8 TRN2 NeuronCores.

Problem: x [4, 4096, 1024] f32, Wq/Wk/Wv [1024, 128] f32 ->
out [4, 4096, 128] f32 (causal softmax(QK^T/sqrt(128)) V).

Sharding: 2 cores per batch element; within a batch element the two
cores take interleaved query rows (even / odd). Each q tile of 128
interleaved rows spans 256 original rows, so tile j needs the kv prefix
0..256(j+1) — identical structure on every core (SPMD), causal waste
only ~6%.

Streaming structure (per core, all matmuls bf16, fp32 PSUM):
  loop over 4 column-pairs (1024 seq cols each):
    DMA xT/xqT slices for this pair (split across both HWDGE rings),
    K.T blocks  = sum_c Wk_c.T @ xT_c        (Wq' pre-scaled 1/sqrt(h))
    V tiles     = sum_c xT_c,t.T @ Wv_c  (ones column appended)
    Q.T blocks  = sum_c Wq'_c.T @ xqT_c
    attention tiles j = 4P..4P+3:
      S.T chunks = (K.T chunk).T @ Q.T tile -> PSUM,
      P.T = exp(S.T) (ScalarE, no max subtraction - scores are O(5)),
      0/1 mask multiply on the two diagonal chunks,
      AV: out += (P.T chunk).T @ [V | 1], software-pipelined one group
      behind the scores so ScalarE exp latency hides under PE work;
      last column is the softmax denominator (DVE reciprocal + scale).
"""
import sys

if "/opt/trn_rl_repo" not in sys.path:
    sys.path.insert(0, "/opt/trn_rl_repo")

import numpy as np
import ml_dtypes

SEQ, D, H = 4096, 1024, 128
NQ = SEQ // 2        # q rows per core
NT = NQ // 128       # q tiles per core (16)
DC = D // 128        # d_model chunks (8)
SB = 512             # seq block for K/Q psum regions
PAIR = 1024          # streaming granularity (seq cols)
N_CORES = 8

_BF16 = ml_dtypes.bfloat16
_CACHED_NC = None


def _build_nc():
    import concourse.bass as bass
    import concourse.bacc as bacc
    import concourse.mybir as mybir
    import concourse.tile as tile

    bf = mybir.dt.bfloat16
    f32 = mybir.dt.float32
    Exp = mybir.ActivationFunctionType.Exp

    nc = bacc.Bacc("TRN2", target_bir_lowering=False, debug=False,
                   num_devices=N_CORES)

    xT_d = nc.dram_tensor("xT", [D, SEQ], bf, kind="ExternalInput")
    xqT_d = nc.dram_tensor("xqT", [D, NQ], bf, kind="ExternalInput")
    wq_d = nc.dram_tensor("wq", [128, DC * H], bf, kind="ExternalInput")
    wk_d = nc.dram_tensor("wk", [128, DC * H], bf, kind="ExternalInput")
    wv_d = nc.dram_tensor("wv", [128, DC * H], bf, kind="ExternalInput")
    m_d = nc.dram_tensor("masks", [128, 256], bf, kind="ExternalInput")
    out_d = nc.dram_tensor("out", [NQ, H], f32, kind="ExternalOutput")

    with tile.TileContext(nc) as tc:
        with (
            tc.tile_pool(name="const", bufs=1) as constp,
            tc.tile_pool(name="xbuf", bufs=1) as xbuf,
            tc.tile_pool(name="acts", bufs=1) as acts,
            tc.tile_pool(name="pt", bufs=3) as ptp,
            tc.tile_pool(name="ob", bufs=2) as obp,
            tc.tile_pool(name="rc", bufs=4) as rcp,
            tc.tile_pool(name="psum_kqv", bufs=2, space="PSUM") as psum_kqv,
            tc.tile_pool(name="psum_s", bufs=2, space="PSUM") as psum_s,
            tc.tile_pool(name="psum_o", bufs=2, space="PSUM") as psum_o,
        ):
            # ---- constants (scalar ring first; sync ring gets xT first) ----
            m_sb = constp.tile([128, 256], bf)
            nc.scalar.dma_start(m_sb[:], m_d[:, :])
            wk_sb = constp.tile([128, DC, H], bf)
            wq_sb = constp.tile([128, DC, H], bf)
            wv_sb = constp.tile([128, DC, H], bf)
            nc.scalar.dma_start(wk_sb[:], wk_d[:, :])
            nc.scalar.dma_start(wv_sb[:], wv_d[:, :])
            nc.scalar.dma_start(wq_sb[:], wq_d[:, :])
            m0 = m_sb[:, 0:128]
            m1 = m_sb[:, 128:256]

            # preload the ACT exp table while DMAs stream
            warm = rcp.tile([128, 1], f32)
            nc.scalar.activation(warm[:], m_sb[:, 0:1], Exp)

            # PE warmup: ~40 throwaway matmuls on the mask tile so the HAM
            # clock-gate opens before the real work arrives
            pw = psum_s.tile([128, 1024], f32, tag="ps")
            for _ in range(40):
                nc.tensor.matmul(pw[:, 0:128], m0, m1, start=True, stop=True)

            xT = xbuf.tile([128, DC, SEQ], bf)
            xqT = xbuf.tile([128, DC, NQ], bf)
            KT = acts.tile([128, SEQ], bf)
            QT = acts.tile([128, NQ], bf)
            V = acts.tile([128, SEQ // 128, H + 1], bf)
            nc.vector.memset(V[:, :, H:H + 1], 1.0)

            for P in range(SEQ // PAIR):
                lo = PAIR * P
                for c in range(DC):
                    eng = nc.sync if c % 2 == 0 else nc.scalar
                    eng.dma_start(xT[:, c, lo:lo + PAIR],
                                  xT_d[128 * c:128 * (c + 1), lo:lo + PAIR])
                if P < 2:
                    for c in range(DC):
                        eng = nc.scalar if c % 2 == 0 else nc.sync
                        eng.dma_start(xqT[:, c, lo:lo + PAIR],
                                      xqT_d[128 * c:128 * (c + 1), lo:lo + PAIR])

                # K.T blocks
                for s in (2 * P, 2 * P + 1):
                    b0 = SB * s
                    pk = psum_kqv.tile([128, SB], f32, tag="kqv")
                    for c in range(DC):
                        nc.tensor.matmul(pk[:], wk_sb[:, c, :],
                                         xT[:, c, b0:b0 + SB],
                                         start=(c == 0), stop=(c == DC - 1))
                    nc.vector.tensor_copy(KT[:, b0:b0 + SB], pk[:])
                # V tiles
                for t in range(8 * P, 8 * P + 8):
                    pv = psum_kqv.tile([128, SB], f32, tag="kqv")
                    for c in range(DC):
                        nc.tensor.matmul(pv[:, 0:H],
                                         xT[:, c, 128 * t:128 * (t + 1)],
                                         wv_sb[:, c, :],
                                         start=(c == 0), stop=(c == DC - 1))
                    nc.vector.tensor_copy(V[:, t, 0:H], pv[:, 0:H])
                # Q.T blocks
                if P < 2:
                    for s in (2 * P, 2 * P + 1):
                        b0 = SB * s
                        pq = psum_kqv.tile([128, SB], f32, tag="kqv")
                        for c in range(DC):
                            nc.tensor.matmul(pq[:], wq_sb[:, c, :],
                                             xqT[:, c, b0:b0 + SB],
                                             start=(c == 0), stop=(c == DC - 1))
                        nc.vector.tensor_copy(QT[:, b0:b0 + SB], pq[:])

                # attention tiles for this pair, AV deferred one group
                for j in range(4 * P, 4 * P + 4):
                    Cj = 2 * (j + 1)
                    po = psum_o.tile([128, H + 1], f32)
                    prev = None
                    for g in range(0, Cj, 8):
                        gn = min(8, Cj - g)
                        ps = psum_s.tile([128, 1024], f32)
                        for k in range(gn):
                            c = g + k
                            nc.tensor.matmul(ps[:, 128 * k:128 * (k + 1)],
                                             KT[:, 128 * c:128 * (c + 1)],
                                             QT[:, 128 * j:128 * (j + 1)],
                                             start=True, stop=True)
                        pt = ptp.tile([128, 1024], bf)
                        nc.scalar.activation(pt[:, 0:128 * gn],
                                             ps[:, 0:128 * gn], Exp)
                        for k in range(gn):
                            c = g + k
                            if c == Cj - 2:
                                nc.vector.tensor_mul(
                                    pt[:, 128 * k:128 * (k + 1)],
                                    pt[:, 128 * k:128 * (k + 1)], m0)
                            elif c == Cj - 1:
                                nc.vector.tensor_mul(
                                    pt[:, 128 * k:128 * (k + 1)],
                                    pt[:, 128 * k:128 * (k + 1)], m1)
                        if prev is not None:
                            pg, pgn, ppt = prev
                            for k in range(pgn):
                                c = pg + k
                                nc.tensor.matmul(po[:],
                                                 ppt[:, 128 * k:128 * (k + 1)],
                                                 V[:, c, :],
                                                 start=(c == 0),
                                                 stop=(c == Cj - 1),
                                                 skip_group_check=True)
                        prev = (g, gn, pt)
                    pg, pgn, ppt = prev
                    for k in range(pgn):
                        c = pg + k
                        nc.tensor.matmul(po[:], ppt[:, 128 * k:128 * (k + 1)],
                                         V[:, c, :],
                                         start=(c == 0), stop=(c == Cj - 1),
                                         skip_group_check=True)
                    rc = rcp.tile([128, 1], f32)
                    nc.vector.reciprocal(rc[:], po[:, H:H + 1])
                    ob = obp.tile([128, H], f32)
                    nc.vector.tensor_scalar_mul(ob[:], po[:, 0:H], rc[:])
                    nc.scalar.dma_start(out_d[128 * j:128 * (j + 1), :], ob[:])

    nc.compile()
    return nc


def _get_nc():
    global _CACHED_NC
    if _CACHED_NC is None:
        _CACHED_NC = _build_nc()
    return _CACHED_NC


def _pack_w(w):
    # [1024, 128] -> [128, 8*128] so SBUF tile [128, c, h] is one linear DMA
    return np.ascontiguousarray(
        w.reshape(DC, 128, H).transpose(1, 0, 2).reshape(128, DC * H))


def _make_in_maps(x, Wq, Wk, Wv):
    scale = np.float32(1.0 / np.sqrt(H))
    wq = _pack_w((np.asarray(Wq, np.float32) * scale).astype(_BF16))
    wk = _pack_w(np.asarray(Wk, np.float32).astype(_BF16))
    wv = _pack_w(np.asarray(Wv, np.float32).astype(_BF16))
    cc, ii = np.meshgrid(np.arange(128), np.arange(128), indexing="ij")
    masks = []
    for p in (0, 1):
        m0 = (cc <= 2 * ii + p).astype(np.float32)
        m1 = (cc + 128 <= 2 * ii + p).astype(np.float32)
        masks.append(np.concatenate([m0, m1], axis=1).astype(_BF16))
    xTs = [np.ascontiguousarray(np.asarray(x[b], np.float32).T).astype(_BF16)
           for b in range(4)]
    in_maps = []
    for core in range(N_CORES):
        b, p = core // 2, core % 2
        xqT = np.ascontiguousarray(np.asarray(x[b, p::2], np.float32).T).astype(_BF16)
        in_maps.append({
            "xT": xTs[b], "xqT": xqT,
            "wq": wq, "wk": wk, "wv": wv,
            "masks": masks[p],
        })
    return in_maps


def _run(x, Wq, Wk, Wv, trace=False):
    from concourse import bass_utils
    nc = _get_nc()
    in_maps = _make_in_maps(x, Wq, Wk, Wv)
    res = bass_utils.run_bass_kernel_spmd(
        nc, in_maps, core_ids=list(range(N_CORES)), trace=trace)
    out = np.empty((4, SEQ, H), np.float32)
    for core in range(N_CORES):
        out[core // 2, core % 2::2] = res.results[core]["out"]
    return out, res


def kernel(x, Wq, Wk, Wv):
    out, _ = _run(x, Wq, Wk, Wv, trace=False)
    return out


# revision 7
# speedup vs baseline: 1.2811x; 1.0136x over previous
"""Causal single-head attention on 8 TRN2 NeuronCores.

Problem: x [4, 4096, 1024] f32, Wq/Wk/Wv [1024, 128] f32 ->
out [4, 4096, 128] f32 (causal softmax(QK^T/sqrt(128)) V).

Sharding: 2 cores per batch element; within a batch element the two
cores take interleaved query rows (even / odd). Each q tile of 128
interleaved rows spans 256 original rows, so tile j needs the kv prefix
0..256(j+1) — identical structure on every core (SPMD), causal waste
only ~6%.

SPMD parity trick: odd-parity cores receive xT with kv columns
pair-swapped (host-side permutation), so "my query rows" are always the
EVEN columns of xT — the Q projection reads xT with a stride-2 access
pattern that is identical on every core, and the causal masks (host
inputs) absorb the permutation. Attention is a sum over kv, so the kv
ordering is irrelevant elsewhere.

Streaming structure (per core, all matmuls bf16, fp32 PSUM):
  loop over 4 column-pairs (1024 seq cols each):
    DMA xT slices for this pair (split across both HWDGE rings),
    K.T blocks  = sum_c Wk_c.T @ xT_c
    V tiles     = sum_c xT_c,t.T @ Wv_c  (ones column appended)
    Q.T block   = sum_c Wq'_c.T @ xT_c[even cols]   (Wq' scaled 1/sqrt h)
    attention tiles j = 4P..4P+3:
      S.T chunks = (K.T chunk).T @ Q.T tile -> PSUM,
      P.T = exp(S.T) (ScalarE, no max subtraction - scores are O(5)),
      0/1 mask multiply on the two diagonal chunks,
      AV: out += (P.T chunk).T @ [V | 1], software-pipelined one group
      behind the scores so ScalarE exp latency hides under PE work;
      last column is the softmax denominator (DVE reciprocal + scale).
"""
import sys

if "/opt/trn_rl_repo" not in sys.path:
    sys.path.insert(0, "/opt/trn_rl_repo")

import numpy as np
import ml_dtypes

SEQ, D, H = 4096, 1024, 128
NQ = SEQ // 2        # q rows per core
NT = NQ // 128       # q tiles per core (16)
DC = D // 128        # d_model chunks (8)
SB = 512             # seq block for K psum regions
PAIR = 1024          # streaming granularity (seq cols)
N_CORES = 8

_BF16 = ml_dtypes.bfloat16
_CACHED_NC = None


def _build_nc():
    import concourse.bass as bass
    import concourse.bacc as bacc
    import concourse.mybir as mybir
    import concourse.tile as tile

    bf = mybir.dt.bfloat16
    f32 = mybir.dt.float32
    Exp = mybir.ActivationFunctionType.Exp

    nc = bacc.Bacc("TRN2", target_bir_lowering=False, debug=False,
                   num_devices=N_CORES)

    xT_d = nc.dram_tensor("xT", [D, SEQ], bf, kind="ExternalInput")
    wq_d = nc.dram_tensor("wq", [128, DC * H], bf, kind="ExternalInput")
    wk_d = nc.dram_tensor("wk", [128, DC * H], bf, kind="ExternalInput")
    wv_d = nc.dram_tensor("wv", [128, DC * H], bf, kind="ExternalInput")
    m_d = nc.dram_tensor("masks", [128, 256], bf, kind="ExternalInput")
    out_d = nc.dram_tensor("out", [NQ, H], f32, kind="ExternalOutput")

    with tile.TileContext(nc) as tc:
        with (
            tc.tile_pool(name="const", bufs=1) as constp,
            tc.tile_pool(name="xbuf", bufs=1) as xbuf,
            tc.tile_pool(name="acts", bufs=1) as acts,
            tc.tile_pool(name="pt", bufs=3) as ptp,
            tc.tile_pool(name="ob", bufs=2) as obp,
            tc.tile_pool(name="rc", bufs=4) as rcp,
            tc.tile_pool(name="psum_kqv", bufs=2, space="PSUM") as psum_kqv,
            tc.tile_pool(name="psum_s", bufs=2, space="PSUM") as psum_s,
            tc.tile_pool(name="psum_o", bufs=2, space="PSUM") as psum_o,
        ):
            # ---- constants (scalar ring first; sync ring gets xT first) ----
            m_sb = constp.tile([128, 256], bf)
            nc.scalar.dma_start(m_sb[:], m_d[:, :])
            wk_sb = constp.tile([128, DC, H], bf)
            wq_sb = constp.tile([128, DC, H], bf)
            wv_sb = constp.tile([128, DC, H], bf)
            nc.scalar.dma_start(wk_sb[:], wk_d[:, :])
            nc.scalar.dma_start(wv_sb[:], wv_d[:, :])
            nc.scalar.dma_start(wq_sb[:], wq_d[:, :])
            m0 = m_sb[:, 0:128]
            m1 = m_sb[:, 128:256]

            # preload the ACT exp table while DMAs stream
            warm = rcp.tile([128, 1], f32)
            nc.scalar.activation(warm[:], m_sb[:, 0:1], Exp)

            # PE warmup: throwaway matmuls on the mask tile so the HAM
            # clock-gate opens before the real work arrives
            pw = psum_s.tile([128, 1024], f32, tag="ps")
            for _ in range(40):
                nc.tensor.matmul(pw[:, 0:128], m0, m1, start=True, stop=True)

            xT = xbuf.tile([128, DC, SEQ], bf)
            KT = acts.tile([128, SEQ], bf)
            QT = acts.tile([128, NQ], bf)
            V = acts.tile([128, SEQ // 128, H + 1], bf)
            nc.vector.memset(V[:, :, H:H + 1], 1.0)

            for P in range(SEQ // PAIR):
                lo = PAIR * P
                for c in range(DC):
                    eng = nc.sync if c % 2 == 0 else nc.scalar
                    eng.dma_start(xT[:, c, lo:lo + PAIR],
                                  xT_d[128 * c:128 * (c + 1), lo:lo + PAIR])

                # K.T blocks
                for s in (2 * P, 2 * P + 1):
                    b0 = SB * s
                    pk = psum_kqv.tile([128, SB], f32, tag="kqv")
                    for c in range(DC):
                        nc.tensor.matmul(pk[:], wk_sb[:, c, :],
                                         xT[:, c, b0:b0 + SB],
                                         start=(c == 0), stop=(c == DC - 1))
                    nc.vector.tensor_copy(KT[:, b0:b0 + SB], pk[:])
                # V tiles
                for t in range(8 * P, 8 * P + 8):
                    pv = psum_kqv.tile([128, SB], f32, tag="kqv")
                    for c in range(DC):
                        nc.tensor.matmul(pv[:, 0:H],
                                         xT[:, c, 128 * t:128 * (t + 1)],
                                         wv_sb[:, c, :],
                                         start=(c == 0), stop=(c == DC - 1))
                    nc.vector.tensor_copy(V[:, t, 0:H], pv[:, 0:H])
                # Q.T block for this pair: even xT columns (stride 2)
                pq = psum_kqv.tile([128, SB], f32, tag="kqv")
                for c in range(DC):
                    nc.tensor.matmul(pq[:], wq_sb[:, c, :],
                                     xT[:, c, lo:lo + PAIR:2],
                                     start=(c == 0), stop=(c == DC - 1))
                nc.vector.tensor_copy(QT[:, SB * P:SB * (P + 1)], pq[:])

                # attention tiles for this pair, AV deferred one group
                for j in range(4 * P, 4 * P + 4):
                    Cj = 2 * (j + 1)
                    po = psum_o.tile([128, H + 1], f32)
                    prev = None
                    for g in range(0, Cj, 8):
                        gn = min(8, Cj - g)
                        ps = psum_s.tile([128, 1024], f32)
                        for k in range(gn):
                            c = g + k
                            nc.tensor.matmul(ps[:, 128 * k:128 * (k + 1)],
                                             KT[:, 128 * c:128 * (c + 1)],
                                             QT[:, 128 * j:128 * (j + 1)],
                                             start=True, stop=True)
                        pt = ptp.tile([128, 1024], bf)
                        nc.scalar.activation(pt[:, 0:128 * gn],
                                             ps[:, 0:128 * gn], Exp)
                        for k in range(gn):
                            c = g + k
                            if c == Cj - 2:
                                nc.vector.tensor_mul(
                                    pt[:, 128 * k:128 * (k + 1)],
                                    pt[:, 128 * k:128 * (k + 1)], m0)
                            elif c == Cj - 1:
                                nc.vector.tensor_mul(
                                    pt[:, 128 * k:128 * (k + 1)],
                                    pt[:, 128 * k:128 * (k + 1)], m1)
                        if prev is not None:
                            pg, pgn, ppt = prev
                            for k in range(pgn):
                                c = pg + k
                                nc.tensor.matmul(po[:],
                                                 ppt[:, 128 * k:128 * (k + 1)],
                                                 V[:, c, :],
                                                 start=(c == 0),
                                                 stop=(c == Cj - 1),
                                                 skip_group_check=True)
                        prev = (g, gn, pt)
                    pg, pgn, ppt = prev
                    for k in range(pgn):
                        c = pg + k
                        nc.tensor.matmul(po[:], ppt[:, 128 * k:128 * (k + 1)],
                                         V[:, c, :],
                                         start=(c == 0), stop=(c == Cj - 1),
                                         skip_group_check=True)
                    rc = rcp.tile([128, 1], f32)
                    nc.vector.reciprocal(rc[:], po[:, H:H + 1])
                    ob = obp.tile([128, H], f32)
                    nc.vector.tensor_scalar_mul(ob[:], po[:, 0:H], rc[:])
                    nc.scalar.dma_start(out_d[128 * j:128 * (j + 1), :], ob[:])

    nc.compile()
    return nc


def _get_nc():
    global _CACHED_NC
    if _CACHED_NC is None:
        _CACHED_NC = _build_nc()
    return _CACHED_NC


def _pack_w(w):
    # [1024, 128] -> [128, 8*128] so SBUF tile [128, c, h] is one linear DMA
    return np.ascontiguousarray(
        w.reshape(DC, 128, H).transpose(1, 0, 2).reshape(128, DC * H))


def _make_in_maps(x, Wq, Wk, Wv):
    scale = np.float32(1.0 / np.sqrt(H))
    wq = _pack_w((np.asarray(Wq, np.float32) * scale).astype(_BF16))
    wk = _pack_w(np.asarray(Wk, np.float32).astype(_BF16))
    wv = _pack_w(np.asarray(Wv, np.float32).astype(_BF16))
    cc, ii = np.meshgrid(np.arange(128), np.arange(128), indexing="ij")
    masks = []
    for p in (0, 1):
        sig = cc + 1 - 2 * (cc % 2) if p == 1 else cc
        m0 = (sig <= 2 * ii + p).astype(np.float32)
        m1 = (sig + 128 <= 2 * ii + p).astype(np.float32)
        masks.append(np.concatenate([m0, m1], axis=1).astype(_BF16))
    in_maps = []
    for core in range(N_CORES):
        b, p = core // 2, core % 2
        xb = np.asarray(x[b], np.float32)
        if p == 1:
            # pair-swap kv rows so this core's q rows are the even ones
            xb = xb.reshape(SEQ // 2, 2, D)[:, ::-1, :].reshape(SEQ, D)
        xT = np.ascontiguousarray(xb.T).astype(_BF16)
        in_maps.append({
            "xT": xT,
            "wq": wq, "wk": wk, "wv": wv,
            "masks": masks[p],
        })
    return in_maps


def _run(x, Wq, Wk, Wv, trace=False):
    from concourse import bass_utils
    nc = _get_nc()
    in_maps = _make_in_maps(x, Wq, Wk, Wv)
    res = bass_utils.run_bass_kernel_spmd(
        nc, in_maps, core_ids=list(range(N_CORES)), trace=trace)
    out = np.empty((4, SEQ, H), np.float32)
    for core in range(N_CORES):
        out[core // 2, core % 2::2] = res.results[core]["out"]
    return out, res


def kernel(x, Wq, Wk, Wv):
    out, _ = _run(x, Wq, Wk, Wv, trace=False)
    return out


# revision 8
# speedup vs baseline: 1.3314x; 1.0393x over previous
"""Causal single-head attention on 8 TRN2 NeuronCores.

Problem: x [4, 4096, 1024] f32, Wq/Wk/Wv [1024, 128] f32 ->
out [4, 4096, 128] f32 (causal softmax(QK^T/sqrt(128)) V).

Sharding: 2 cores per batch element; within a batch element the two
cores take interleaved query rows (even / odd). Each q tile of 128
interleaved rows spans 256 original rows, so tile j needs the kv prefix
0..256(j+1) — identical structure on every core (SPMD), causal waste
only ~6%.

SPMD parity trick: odd-parity cores receive xT with kv columns
pair-swapped (host-side permutation), so "my query rows" are always the
EVEN columns of xT — the Q projection reads xT with a stride-2 access
pattern that is identical on every core, and the causal masks (host
inputs) absorb the permutation. Attention is a sum over kv, so the kv
ordering is irrelevant elsewhere.

Streaming structure (per core, all matmuls bf16, fp32 PSUM):
  loop over 4 column-pairs (1024 seq cols each):
    DMA xT slices for this pair (split across both HWDGE rings),
    K.T blocks  = sum_c Wk_c.T @ xT_c
    V tiles     = sum_c xT_c,t.T @ Wv_c  (ones column appended)
    Q.T block   = sum_c Wq'_c.T @ xT_c[even cols]   (Wq' scaled 1/sqrt h)
    attention tiles j = 4P..4P+3:
      S.T chunks = (K.T chunk).T @ Q.T tile -> PSUM,
      P.T = exp(S.T) (ScalarE, no max subtraction - scores are O(5)),
      0/1 mask multiply on the two diagonal chunks,
      AV: out += (P.T chunk).T @ [V | 1], software-pipelined one group
      behind the scores so ScalarE exp latency hides under PE work;
      last column is the softmax denominator (DVE reciprocal + scale).
"""
import sys

if "/opt/trn_rl_repo" not in sys.path:
    sys.path.insert(0, "/opt/trn_rl_repo")

import numpy as np
import ml_dtypes

SEQ, D, H = 4096, 1024, 128
NQ = SEQ // 2        # q rows per core
NT = NQ // 128       # q tiles per core (16)
DC = D // 128        # d_model chunks (8)
SB = 512             # seq block for K psum regions
PAIR = 1024          # streaming granularity (seq cols)
N_CORES = 8

_BF16 = ml_dtypes.bfloat16
_CACHED_NC = None


def _build_nc():
    import concourse.bass as bass
    import concourse.bacc as bacc
    import concourse.mybir as mybir
    import concourse.tile as tile

    bf = mybir.dt.bfloat16
    f32 = mybir.dt.float32
    Exp = mybir.ActivationFunctionType.Exp

    nc = bacc.Bacc("TRN2", target_bir_lowering=False, debug=False,
                   num_devices=N_CORES)

    xT_d = nc.dram_tensor("xT", [D, SEQ], bf, kind="ExternalInput")
    wq_d = nc.dram_tensor("wq", [128, DC * H], bf, kind="ExternalInput")
    wk_d = nc.dram_tensor("wk", [128, DC * H], bf, kind="ExternalInput")
    wv_d = nc.dram_tensor("wv", [128, DC * H], bf, kind="ExternalInput")
    m_d = nc.dram_tensor("masks", [128, 256], bf, kind="ExternalInput")
    out_d = nc.dram_tensor("out", [NQ, H], f32, kind="ExternalOutput")

    with tile.TileContext(nc) as tc:
        with (
            tc.tile_pool(name="const", bufs=1) as constp,
            tc.tile_pool(name="xbuf", bufs=1) as xbuf,
            tc.tile_pool(name="acts", bufs=1) as acts,
            tc.tile_pool(name="pt", bufs=3) as ptp,
            tc.tile_pool(name="ob", bufs=2) as obp,
            tc.tile_pool(name="rc", bufs=4) as rcp,
            tc.tile_pool(name="psum_kqv", bufs=2, space="PSUM") as psum_kqv,
            tc.tile_pool(name="psum_s", bufs=2, space="PSUM") as psum_s,
            tc.tile_pool(name="psum_o", bufs=2, space="PSUM") as psum_o,
        ):
            # ---- constants (scalar ring first; sync ring gets xT first) ----
            m_sb = constp.tile([128, 256], bf)
            nc.scalar.dma_start(m_sb[:], m_d[:, :])
            wk_sb = constp.tile([128, DC, H], bf)
            wq_sb = constp.tile([128, DC, H], bf)
            wv_sb = constp.tile([128, DC, H], bf)
            nc.scalar.dma_start(wk_sb[:], wk_d[:, :])
            nc.scalar.dma_start(wv_sb[:], wv_d[:, :])
            nc.scalar.dma_start(wq_sb[:], wq_d[:, :])
            m0 = m_sb[:, 0:128]
            m1 = m_sb[:, 128:256]

            # PE warmup: throwaway matmuls on a memset tile (no DMA
            # dependency) so the HAM clock-gate opens and PE stays warm
            # until the first xT data lands
            wtile = constp.tile([128, 640], bf)
            nc.vector.memset(wtile[:], 0.25)
            pw = psum_s.tile([128, 1024], f32, tag="ps")
            for _ in range(34):
                nc.tensor.matmul(pw[:, 0:512], wtile[:, 0:128],
                                 wtile[:, 128:640], start=True, stop=True)

            # preload the ACT exp table while DMAs stream
            warm = rcp.tile([128, 1], f32)
            nc.scalar.activation(warm[:], wtile[:, 0:1], Exp)

            xT = xbuf.tile([128, DC, SEQ], bf)
            KT = acts.tile([128, SEQ], bf)
            QT = acts.tile([128, NQ], bf)
            V = acts.tile([128, SEQ // 128, H + 1], bf)
            nc.vector.memset(V[:, :, H:H + 1], 1.0)

            # all xT DMAs up-front: both HWDGE rings crunch through them
            # back-to-back, decoupled from the compute emission order
            for P in range(SEQ // PAIR):
                lo = PAIR * P
                for c in range(DC):
                    eng = nc.sync if c % 2 == 0 else nc.scalar
                    eng.dma_start(xT[:, c, lo:lo + PAIR],
                                  xT_d[128 * c:128 * (c + 1), lo:lo + PAIR])

            for P in range(SEQ // PAIR):
                lo = PAIR * P
                # K.T blocks
                for s in (2 * P, 2 * P + 1):
                    b0 = SB * s
                    pk = psum_kqv.tile([128, SB], f32, tag="kqv")
                    for c in range(DC):
                        nc.tensor.matmul(pk[:], wk_sb[:, c, :],
                                         xT[:, c, b0:b0 + SB],
                                         start=(c == 0), stop=(c == DC - 1))
                    nc.vector.tensor_copy(KT[:, b0:b0 + SB], pk[:])
                # V tiles
                for t in range(8 * P, 8 * P + 8):
                    pv = psum_kqv.tile([128, SB], f32, tag="kqv")
                    for c in range(DC):
                        nc.tensor.matmul(pv[:, 0:H],
                                         xT[:, c, 128 * t:128 * (t + 1)],
                                         wv_sb[:, c, :],
                                         start=(c == 0), stop=(c == DC - 1))
                    nc.vector.tensor_copy(V[:, t, 0:H], pv[:, 0:H])
                # Q.T block for this pair: even xT columns (stride 2)
                pq = psum_kqv.tile([128, SB], f32, tag="kqv")
                for c in range(DC):
                    nc.tensor.matmul(pq[:], wq_sb[:, c, :],
                                     xT[:, c, lo:lo + PAIR:2],
                                     start=(c == 0), stop=(c == DC - 1))
                nc.vector.tensor_copy(QT[:, SB * P:SB * (P + 1)], pq[:])

                # attention tiles for this pair, AV deferred one group
                for j in range(4 * P, 4 * P + 4):
                    Cj = 2 * (j + 1)
                    po = psum_o.tile([128, H + 1], f32)
                    prev = None
                    for g in range(0, Cj, 8):
                        gn = min(8, Cj - g)
                        ps = psum_s.tile([128, 1024], f32)
                        for k in range(gn):
                            c = g + k
                            nc.tensor.matmul(ps[:, 128 * k:128 * (k + 1)],
                                             KT[:, 128 * c:128 * (c + 1)],
                                             QT[:, 128 * j:128 * (j + 1)],
                                             start=True, stop=True)
                        pt = ptp.tile([128, 1024], bf)
                        nc.scalar.activation(pt[:, 0:128 * gn],
                                             ps[:, 0:128 * gn], Exp)
                        for k in range(gn):
                            c = g + k
                            if c == Cj - 2:
                                nc.vector.tensor_mul(
                                    pt[:, 128 * k:128 * (k + 1)],
                                    pt[:, 128 * k:128 * (k + 1)], m0)
                            elif c == Cj - 1:
                                nc.vector.tensor_mul(
                                    pt[:, 128 * k:128 * (k + 1)],
                                    pt[:, 128 * k:128 * (k + 1)], m1)
                        if prev is not None:
                            pg, pgn, ppt = prev
                            for k in range(pgn):
                                c = pg + k
                                nc.tensor.matmul(po[:],
                                                 ppt[:, 128 * k:128 * (k + 1)],
                                                 V[:, c, :],
                                                 start=(c == 0),
                                                 stop=(c == Cj - 1),
                                                 skip_group_check=True)
                        prev = (g, gn, pt)
                    pg, pgn, ppt = prev
                    for k in range(pgn):
                        c = pg + k
                        nc.tensor.matmul(po[:], ppt[:, 128 * k:128 * (k + 1)],
                                         V[:, c, :],
                                         start=(c == 0), stop=(c == Cj - 1),
                                         skip_group_check=True)
                    rc = rcp.tile([128, 1], f32)
                    nc.vector.reciprocal(rc[:], po[:, H:H + 1])
                    ob = obp.tile([128, H], f32)
                    nc.vector.tensor_scalar_mul(ob[:], po[:, 0:H], rc[:])
                    nc.scalar.dma_start(out_d[128 * j:128 * (j + 1), :], ob[:])

    nc.compile()
    return nc


def _get_nc():
    global _CACHED_NC
    if _CACHED_NC is None:
        _CACHED_NC = _build_nc()
    return _CACHED_NC


def _pack_w(w):
    # [1024, 128] -> [128, 8*128] so SBUF tile [128, c, h] is one linear DMA
    return np.ascontiguousarray(
        w.reshape(DC, 128, H).transpose(1, 0, 2).reshape(128, DC * H))


def _make_in_maps(x, Wq, Wk, Wv):
    scale = np.float32(1.0 / np.sqrt(H))
    wq = _pack_w((np.asarray(Wq, np.float32) * scale).astype(_BF16))
    wk = _pack_w(np.asarray(Wk, np.float32).astype(_BF16))
    wv = _pack_w(np.asarray(Wv, np.float32).astype(_BF16))
    cc, ii = np.meshgrid(np.arange(128), np.arange(128), indexing="ij")
    masks = []
    for p in (0, 1):
        sig = cc + 1 - 2 * (cc % 2) if p == 1 else cc
        m0 = (sig <= 2 * ii + p).astype(np.float32)
        m1 = (sig + 128 <= 2 * ii + p).astype(np.float32)
        masks.append(np.concatenate([m0, m1], axis=1).astype(_BF16))
    in_maps = []
    for core in range(N_CORES):
        b, p = core // 2, core % 2
        xb = np.asarray(x[b], np.float32)
        if p == 1:
            # pair-swap kv rows so this core's q rows are the even ones
            xb = xb.reshape(SEQ // 2, 2, D)[:, ::-1, :].reshape(SEQ, D)
        xT = np.ascontiguousarray(xb.T).astype(_BF16)
        in_maps.append({
            "xT": xT,
            "wq": wq, "wk": wk, "wv": wv,
            "masks": masks[p],
        })
    return in_maps


def _run(x, Wq, Wk, Wv, trace=False):
    from concourse import bass_utils
    nc = _get_nc()
    in_maps = _make_in_maps(x, Wq, Wk, Wv)
    res = bass_utils.run_bass_kernel_spmd(
        nc, in_maps, core_ids=list(range(N_CORES)), trace=trace)
    out = np.empty((4, SEQ, H), np.float32)
    for core in range(N_CORES):
        out[core // 2, core % 2::2] = res.results[core]["out"]
    return out, res


def kernel(x, Wq, Wk, Wv):
    out, _ = _run(x, Wq, Wk, Wv, trace=False)
    return out
